# revision 1
# baseline (speedup 1.0000x reference)
"""Trainium2 Bass kernel for MultiHeadLatentAttention.

Reference computation (B=2, S=2048, HIDDEN=2048, 16 heads x 128, LATENT=512):
  q_lat = x @ Wq_d ; kv_lat = x @ Wkv_d
  q = split_heads(q_lat @ Wq_u) ; k = split_heads(kv_lat @ Wk_u) ; v = split_heads(kv_lat @ Wv_u)
  q, k = rope(q, k)
  out = softmax(causal(q k^T / sqrt(d))) @ v   -> merge heads -> @ Wo

Sharding: 8 cores = 2 batches (data parallel) x 4-way tensor parallel over
heads (4 heads/core).  Each core computes the full latents for its batch
(replicated within the 4-core group), the up-projections + attention for its
4 heads, and a partial output projection over its heads' slice of Wo's input
dim.  The host sums the 4 partials per batch (cheap elementwise add).

Dataflow on-core is fully transposed ([feature, seq] layout) so no PE
transposes are needed anywhere:
  latT = Wd^T xT -> qT/kT per head via up-proj; rotate_half for rope is a
  single signed-permutation matmul on the PE; v in [seq, d] layout;
  scoresT[k, q] = kT-block-stationary x qT-moving; exp on ACT; softmax
  denominators via ones-vector matmuls accumulated on the PE; AV accumulated
  as v^T-stationary x expT; 1/denominator applied on the attention output
  (PSUM->SBUF copy fused); final Wo stage back in [seq, out] orientation.
  Causal structure skips above-diagonal blocks and narrows partial blocks.

Matmuls run in float32r (full PE rate; fp32 is 1/4 rate), fp32 accumulation.
"""

import sys
from contextlib import ExitStack

sys.path.insert(0, "/opt/trn_rl_repo")

import numpy as np

import concourse.bass as bass
import concourse.mybir as mybir
import concourse.tile as tile
from concourse import bacc
from concourse.bass_utils import run_bass_kernel_spmd

HIDDEN = 2048
LATENT = 512
NUM_HEADS = 16
HEAD_DIM = 128
THETA = 10000.0
B = 2
S_FULL = 2048
N_CORES = 8
TP = 4  # tensor-parallel group size (heads 16 / 4 = 4 per core)
HPC = NUM_HEADS // TP  # heads per core
DSL = HPC * HEAD_DIM  # per-core head-dim slice width (512)

F32 = mybir.dt.float32
F32R = mybir.dt.float32r

NEG = -1.0e30
SCALE = 1.0 / np.sqrt(HEAD_DIM)


def build_nc(S=S_FULL, finalize=True, iters=1, stages="full", variant="",
             gather=False):
    """Build the single-core SPMD program (same program all 8 cores).

    iters > 1 wraps the whole body in an on-device repeat loop (timing rig).
    """
    nc = bacc.Bacc(None, target_bir_lowering=False)

    KC_H = HIDDEN // 128   # 16 contraction chunks for hidden dim
    KC_L = LATENT // 128   # 4 contraction chunks for latent dim
    NB = S // 512          # number of 512-wide seq blocks
    SC = S // 128          # number of 128-wide seq chunks
    NH = S // 256          # number of 256-wide seq half-blocks (stage A)

    SQ = S // TP if gather else S
    NHL = SQ // 256 if gather else S // 256  # local half-blocks in stage A
    xT = nc.dram_tensor("xT", [HIDDEN, SQ], F32R, kind="ExternalInput")
    latpq = nc.dram_tensor("latpq", [128, LATENT // 128, SQ], F32R)
    latpkv = nc.dram_tensor("latpkv", [128, LATENT // 128, SQ], F32R)
    gq = nc.dram_tensor("gq", [TP, 128, LATENT // 128, SQ], F32R)
    gkv = nc.dram_tensor("gkv", [TP, 128, LATENT // 128, SQ], F32R)
    wqd = nc.dram_tensor("wqd", [HIDDEN, LATENT], F32R, kind="ExternalInput")
    wkvd = nc.dram_tensor("wkvd", [HIDDEN, LATENT], F32R, kind="ExternalInput")
    wqu = nc.dram_tensor("wqu", [LATENT, DSL], F32R, kind="ExternalInput")
    wku = nc.dram_tensor("wku", [LATENT, DSL], F32R, kind="ExternalInput")
    wvu = nc.dram_tensor("wvu", [LATENT, DSL], F32R, kind="ExternalInput")
    wo = nc.dram_tensor("wo", [DSL, HIDDEN], F32R, kind="ExternalInput")
    cosd = nc.dram_tensor("cosd", [128, S], F32, kind="ExternalInput")
    sind = nc.dram_tensor("sind", [128, S], F32, kind="ExternalInput")
    mtd = nc.dram_tensor("mtd", [2, 128, 256], F32, kind="ExternalInput")
    permd = nc.dram_tensor("permd", [128, 128], F32R, kind="ExternalInput")
    onesd = nc.dram_tensor("onesd", [128, 1], F32R, kind="ExternalInput")
    y = nc.dram_tensor("y", [S, HIDDEN], F32, kind="ExternalOutput")

    unroll = 1
    if iters < 0:          # negative: unrolled repeats (collective-safe)
        unroll, iters = -iters, 1
    with tile.TileContext(nc) as tc, ExitStack() as _es:
        if iters > 1:
            _es.enter_context(tc.For_i(0, iters, 1))
        for _u in range(unroll):
        # ---- persistent pools (allocated bottom of stack, live long) ----
          with tc.tile_pool(name="p_out", bufs=1) as p_out, \
               tc.tile_pool(name="p_lat", bufs=1) as p_lat, \
               tc.tile_pool(name="p_const", bufs=1) as p_const:

              outT = p_out.tile([128, HPC, S], F32R)      # attention out, transposed
              latq = p_lat.tile([128, KC_L, S], F32R)     # q_latT
              latkv = p_lat.tile([128, KC_L, S], F32R)    # kv_latT
              mask_sb = p_const.tile([128, 2, 256], F32)
              ones_sb = p_const.tile([128, 1], F32R)
              perm_sb = p_const.tile([128, 128], F32R)

              # ================= stage A: down projections =================
              with tc.tile_pool(name="p_wd", bufs=1) as p_wd, \
                   tc.tile_pool(name="p_xt", bufs=2) as p_xt, \
                   tc.tile_pool(name="p_st", bufs=4) as p_st, \
                   tc.tile_pool(name="ps_a", bufs=4, space="PSUM") as ps_a:
                  wqd_sb = p_wd.tile([128, KC_H, LATENT], F32R)
                  wkvd_sb = p_wd.tile([128, KC_H, LATENT], F32R)

                  def load_w_col(w_sb, w_dram, m):
                      nc.sync.dma_start(
                          out=w_sb[:, :, m * 128:(m + 1) * 128],
                          in_=w_dram.rearrange("(kc p) l -> p kc l", p=128)
                          [:, :, m * 128:(m + 1) * 128])

                  # prefetch order: wqd m=0 first, then slab 0 (in loop), then
                  # the rest, so the PE can start ~15us in.
                  load_w_col(wqd_sb, wqd, 0)
                  for nh in range(NHL):
                      xslab = p_xt.tile([128, KC_H, 256], F32R, tag="xslab")
                      nc.sync.dma_start(
                          out=xslab,
                          in_=xT.rearrange("(kc p) s -> p kc s", p=128)
                          [:, :, nh * 256:(nh + 1) * 256])
                      if nh == 0:
                          load_w_col(wkvd_sb, wkvd, 0)
                          for m in range(1, KC_L):
                              load_w_col(wqd_sb, wqd, m)
                              load_w_col(wkvd_sb, wkvd, m)
                          # constants (needed later; low priority)
                          nc.sync.dma_start(
                              out=mask_sb, in_=mtd.rearrange("j p c -> p j c"))
                          nc.sync.dma_start(out=ones_sb, in_=onesd[:, :])
                          nc.sync.dma_start(out=perm_sb, in_=permd[:, :])
                      _groups = []
                      if "X" not in variant:
                          for m in range(KC_L):
                              _groups.append((wqd_sb, latq, latpq, m))
                              _groups.append((wkvd_sb, latkv, latpkv, m))
                      for w_sb, lat, latp, m in _groups:
                          if True:
                              acc = ps_a.tile([128, 256], F32, tag="acc_a")
                              for kc in range(KC_H):
                                  nc.tensor.matmul(
                                      acc,
                                      w_sb[:, kc, m * 128:(m + 1) * 128],
                                      xslab[:, kc, :],
                                      start=(kc == 0), stop=(kc == KC_H - 1))
                              if gather:
                                  st = p_st.tile([128, 256], F32R, tag="st")
                                  nc.scalar.copy(st, acc)
                                  nc.sync.dma_start(
                                      out=latp[:, m, nh * 256:(nh + 1) * 256],
                                      in_=st)
                              else:
                                  nc.scalar.copy(
                                      lat[:, m, nh * 256:(nh + 1) * 256], acc)
                  if gather:
                      nc.gpsimd.collective_compute(
                          "AllGather", mybir.AluOpType.bypass,
                          replica_groups=[[0, 1, 2, 3], [4, 5, 6, 7]],
                          ins=[latpq[:, :, :]], outs=[gq[:, :, :, :]])
                      nc.gpsimd.collective_compute(
                          "AllGather", mybir.AluOpType.bypass,
                          replica_groups=[[0, 1, 2, 3], [4, 5, 6, 7]],
                          ins=[latpkv[:, :, :]], outs=[gkv[:, :, :, :]])
                      for m in range(KC_L):
                          nc.sync.dma_start(
                              out=latq[:, m, :],
                              in_=gq.rearrange("t p m s -> p m t s")[:, m, :, :])
                          nc.sync.dma_start(
                              out=latkv[:, m, :],
                              in_=gkv.rearrange("t p m s -> p m t s")[:, m, :, :])

              if stages == "a":
                  if "X" in variant:
                      nc.sync.dma_start(out=y[0:128, 0:LATENT],
                                        in_=wqd_sb[:, 0, :].bitcast(F32))
                      nc.sync.dma_start(out=y[128:256, 0:LATENT],
                                        in_=wkvd_sb[:, 0, :].bitcast(F32))
                  else:
                      nc.sync.dma_start(out=y[0:128, 0:S], in_=latq[:, 0, :].bitcast(F32))
                      nc.sync.dma_start(out=y[128:256, 0:S], in_=latkv[:, 0, :].bitcast(F32))
                      nc.sync.dma_start(out=y[256:384, 0:S], in_=latq[:, 2, :].bitcast(F32))
                      nc.sync.dma_start(out=y[384:512, 0:S], in_=latkv[:, 2, :].bitcast(F32))
              # ================= stage B0: v for all 4 heads ===============
              run_b = stages in ("av", "ac", "full")
              run_heads = stages in ("ac", "full")
              run_d = stages == "full"
              with tc.tile_pool(name="p_v", bufs=1) as p_v:
                  with tc.tile_pool(name="p_wv", bufs=1) as p_wv, \
                       tc.tile_pool(name="ps_v", bufs=4, space="PSUM") as ps_v:
                      v_sb = p_v.tile([128, SC, DSL], F32R)
                      wvu_sb = p_wv.tile([128, KC_L, DSL], F32R)
                      if run_b:
                          nc.sync.dma_start(
                              out=wvu_sb,
                              in_=wvu.rearrange("(kc p) d -> p kc d", p=128))
                      for sc in range(SC if run_b else 0):
                          acc = ps_v.tile([128, DSL], F32, tag="acc_v")
                          for kc in range(KC_L):
                              nc.tensor.matmul(
                                  acc,
                                  latkv[:, kc, sc * 128:(sc + 1) * 128],
                                  wvu_sb[:, kc, :],
                                  start=(kc == 0), stop=(kc == KC_L - 1))
                          nc.vector.tensor_copy(v_sb[:, sc, :], acc)

                  if stages == "av":
                      nc.sync.dma_start(out=y[0:128, 0:S], in_=latq[:, 0, :].bitcast(F32))
                      nc.sync.dma_start(out=y[128:256, 0:S], in_=latkv[:, 0, :].bitcast(F32))
                      nc.sync.dma_start(
                          out=y[256:384, 0:DSL],
                          in_=v_sb[:, SC - 1, :].bitcast(F32))
                  # ============ stages B/C per head: up-proj + attention ====
                  with tc.tile_pool(name="p_rope", bufs=1) as p_rope:
                      cos_sb = p_rope.tile([128, S], F32)
                      sin_sb = p_rope.tile([128, S], F32)
                      if run_heads:
                          nc.sync.dma_start(out=cos_sb, in_=cosd[:, :])
                          nc.sync.dma_start(out=sin_sb, in_=sind[:, :])
                      with tc.tile_pool(name="p_rt", bufs=3) as p_rt, \
                           tc.tile_pool(name="p_at", bufs=6) as p_at, \
                           tc.tile_pool(name="p_rb", bufs=2) as p_rb:
                        for h in range(HPC if run_heads else 0):
                          with tc.tile_pool(name="p_head", bufs=1) as p_head, \
                               tc.tile_pool(name="p_wu", bufs=2) as p_wu:
                              qT = p_head.tile([128, S], F32R, tag="qT")
                              kT = p_head.tile([128, S], F32R, tag="kT")
                              wq_sb = p_wu.tile([128, KC_L, 128], F32R, tag="wq")
                              wk_sb = p_wu.tile([128, KC_L, 128], F32R, tag="wk")
                              hs = h * 128
                              nc.sync.dma_start(
                                  out=wq_sb,
                                  in_=wqu.rearrange("(kc p) d -> p kc d", p=128)
                                  [:, :, hs:hs + 128])
                              nc.sync.dma_start(
                                  out=wk_sb,
                                  in_=wku.rearrange("(kc p) d -> p kc d", p=128)
                                  [:, :, hs:hs + 128])

                              with tc.tile_pool(name="ps_b", bufs=2,
                                                space="PSUM") as ps_b, \
                                   tc.tile_pool(name="ps_br", bufs=2,
                                                space="PSUM") as ps_br:
                                for dst, w_sb, lat in (
                                        (qT, wq_sb, latq),
                                        (kT, wk_sb, latkv)):
                                    for nb in range(NB):
                                        sl = slice(nb * 512, (nb + 1) * 512)
                                        pa = ps_b.tile([128, 512], F32, tag="pa")
                                        for kc in range(KC_L):
                                            nc.tensor.matmul(
                                                pa, w_sb[:, kc, :], lat[:, kc, sl],
                                                start=(kc == 0),
                                                stop=(kc == KC_L - 1))
                                        raw = p_rt.tile([128, 512], F32R, tag="raw")
                                        nc.vector.tensor_copy(raw, pa)
                                        pr = ps_br.tile([128, 512], F32, tag="pr")
                                        nc.tensor.matmul(pr, perm_sb, raw,
                                                         start=True, stop=True)
                                        rt = p_rt.tile([128, 512], F32, tag="rt")
                                        nc.vector.tensor_mul(dst[:, sl], pa,
                                                             cos_sb[:, sl])
                                        nc.vector.tensor_mul(rt, pr, sin_sb[:, sl])
                                        nc.vector.tensor_add(dst[:, sl],
                                                             dst[:, sl], rt)

                              # ---- attention for head h ----
                              with tc.tile_pool(name="ps_s", bufs=4,
                                                space="PSUM") as ps_s, \
                                   tc.tile_pool(name="ps_o", bufs=2,
                                                space="PSUM") as ps_o, \
                                   tc.tile_pool(name="ps_n", bufs=2,
                                                space="PSUM") as ps_n:
                                  for qb in range(NB):
                                      kb_hi = 4 * qb + 4
                                      po = ps_o.tile([128, 512], F32, tag="po")
                                      pn = ps_n.tile([1, 512], F32, tag="pn")
                                      for kb in range(kb_hi):
                                          j = kb - 4 * qb
                                          # narrowed q range for partial blocks
                                          # (keep N >= 256 for the f32r rate)
                                          off = min(j, 2) * 128 if j >= 0 else 0
                                          w = 512 - off
                                          q0 = qb * 512 + off
                                          ps = ps_s.tile([128, 512], F32, tag="ps")
                                          nc.tensor.matmul(
                                              ps[:, 0:w],
                                              kT[:, kb * 128:(kb + 1) * 128],
                                              qT[:, q0:q0 + w],
                                              start=True, stop=True)
                                          if j >= 0 and "M" not in variant:
                                              jj = j - off // 128
                                              mw = (jj + 1) * 128
                                              nc.vector.tensor_add(
                                                  ps[:, 0:mw], ps[:, 0:mw],
                                                  mask_sb[:, jj, 0:mw])
                                          et = p_at.tile([128, 512], F32R, tag="et")
                                          if "E" in variant:
                                              nc.vector.tensor_copy(et[:, 0:w],
                                                                    ps[:, 0:w])
                                          else:
                                              nc.scalar.activation(
                                                  out=et[:, 0:w], in_=ps[:, 0:w],
                                                  func=mybir.ActivationFunctionType.Exp,
                                                  scale=float(SCALE))
                                          nc.tensor.matmul(
                                              po[:, off:512],
                                              v_sb[:, kb, hs:hs + 128],
                                              et[:, 0:w],
                                              start=(kb == 0),
                                              stop=(kb == kb_hi - 1))
                                          if "O" not in variant:
                                              nc.tensor.matmul(
                                                  pn[0:1, off:512],
                                                  ones_sb[:, 0:1],
                                                  et[:, 0:w],
                                                  start=(kb == 0),
                                                  stop=(kb == kb_hi - 1))
                                      qsl = slice(qb * 512, (qb + 1) * 512)
                                      rc = p_rb.tile([1, 512], F32, tag="rc")
                                      if "O" in variant:
                                          nc.vector.reciprocal(rc, po[0:1, :])
                                      else:
                                          nc.vector.reciprocal(rc, pn[0:1, :])
                                      rb = p_rb.tile([128, 512], F32, tag="rb")
                                      nc.gpsimd.partition_broadcast(rb, rc)
                                      nc.vector.tensor_mul(outT[:, h, qsl], po, rb)

              if stages == "ac":
                  for h2 in range(HPC):
                      nc.sync.dma_start(out=y[h2 * 128:(h2 + 1) * 128, 0:S],
                                        in_=outT[:, h2, :].bitcast(F32))
              # ================= stage D: output projection ================
              with tc.tile_pool(name="p_wo", bufs=1) as p_wo, \
                   tc.tile_pool(name="p_fin", bufs=3) as p_fin, \
                   tc.tile_pool(name="ps_d", bufs=4, space="PSUM") as ps_d:
                  wo_sb = p_wo.tile([128, HPC, HIDDEN], F32R, tag="wo")
                  if run_d:
                      nc.sync.dma_start(
                          out=wo_sb, in_=wo.rearrange("(ic p) o -> p ic o", p=128))
                  for sc in range(SC if run_d else 0):
                      fin = p_fin.tile([128, HIDDEN], F32, tag="fin")
                      for ob in range(HIDDEN // 512):
                          acc = ps_d.tile([128, 512], F32, tag="acc_d")
                          for ic in range(HPC):
                              nc.tensor.matmul(
                                  acc,
                                  outT[:, ic, sc * 128:(sc + 1) * 128],
                                  wo_sb[:, ic, ob * 512:(ob + 1) * 512],
                                  start=(ic == 0), stop=(ic == HPC - 1))
                          osl = slice(ob * 512, (ob + 1) * 512)
                          if ob % 2 == 0:
                              nc.scalar.copy(fin[:, osl], acc)
                          else:
                              nc.vector.tensor_copy(fin[:, osl], acc)
                      nc.sync.dma_start(
                          out=y[sc * 128:(sc + 1) * 128, :], in_=fin)


    if finalize:
        nc.finalize()
    return nc


# ---------------------------------------------------------------------------
# host-side helpers


def host_inputs(x, Wq_d, Wkv_d, Wq_u, Wk_u, Wv_u, Wo, S=S_FULL, gather=False):
    """Build the 8 per-core input maps from full inputs."""
    x = np.asarray(x, dtype=np.float32)
    Wq_d = np.asarray(Wq_d, dtype=np.float32)
    Wkv_d = np.asarray(Wkv_d, dtype=np.float32)
    Wq_u = np.asarray(Wq_u, dtype=np.float32)
    Wk_u = np.asarray(Wk_u, dtype=np.float32)
    Wv_u = np.asarray(Wv_u, dtype=np.float32)
    Wo = np.asarray(Wo, dtype=np.float32)

    inv_freq = 1.0 / (THETA ** (np.arange(0, HEAD_DIM, 2, dtype=np.float64)
                                / HEAD_DIM))  # (64,)
    pos = np.arange(S, dtype=np.float64)
    ang = pos[None, :] * np.concatenate([inv_freq, inv_freq])[:, None]  # (128, S)
    COS = np.cos(ang).astype(np.float32)
    SIN = np.sin(ang).astype(np.float32)

    # masks for narrowed diagonal blocks: jj=0 -> mask c<r on first 128 cols;
    # jj=1 -> mask c<128+r on first 256 cols
    MT = np.zeros((2, 128, 256), dtype=np.float32)
    r = np.arange(128)[:, None]
    c = np.arange(256)[None, :]
    MT[0] = np.where(c >= r, 0.0, NEG)
    MT[1] = np.where(c >= 128 + r, 0.0, NEG)

    # signed permutation for rotate_half in [d, seq] layout:
    # out[m] = -in[m+64] for m<64 ; +in[m-64] for m>=64
    PERM = np.zeros((128, 128), dtype=np.float32)
    for m in range(64):
        PERM[m + 64, m] = -1.0
        PERM[m, m + 64] = 1.0

    in_maps = []
    for core in range(N_CORES):
        b, tp = core // TP, core % TP
        sl = slice(tp * DSL, (tp + 1) * DSL)
        SQ = S // TP if gather else S
        xt_c = x[b, :S].T
        if gather:
            xt_c = xt_c[:, tp * SQ:(tp + 1) * SQ]
        in_maps.append({
            "xT": np.ascontiguousarray(xt_c),
            "wqd": Wq_d,
            "wkvd": Wkv_d,
            "wqu": np.ascontiguousarray(Wq_u[:, sl]),
            "wku": np.ascontiguousarray(Wk_u[:, sl]),
            "wvu": np.ascontiguousarray(Wv_u[:, sl]),
            "wo": np.ascontiguousarray(Wo[sl, :]),
            "cosd": COS,
            "sind": SIN,
            "mtd": MT,
            "permd": PERM,
            "onesd": np.ones((128, 1), dtype=np.float32),
        })
    return in_maps


def assemble(results, S=S_FULL):
    out = np.zeros((B, S, HIDDEN), dtype=np.float32)
    for core in range(N_CORES):
        out[core // TP] += results[core]["y"]
    return out


_NC_CACHE = {}


def kernel(x, Wq_d, Wkv_d, Wq_u, Wk_u, Wv_u, Wo):
    S = x.shape[1]
    if S not in _NC_CACHE:
        _NC_CACHE[S] = build_nc(S)
    nc = _NC_CACHE[S]
    in_maps = host_inputs(x, Wq_d, Wkv_d, Wq_u, Wk_u, Wv_u, Wo, S=S)

    res = run_bass_kernel_spmd(nc, in_maps, list(range(N_CORES)))
    return assemble(res.results, S=S)



# revision 45
# speedup vs baseline: 1.0434x; 1.0434x over previous
"""Trainium2 Bass kernel for MultiHeadLatentAttention.

Reference computation (B=2, S=2048, HIDDEN=2048, 16 heads x 128, LATENT=512):
  q_lat = x @ Wq_d ; kv_lat = x @ Wkv_d
  q = split_heads(q_lat @ Wq_u) ; k = split_heads(kv_lat @ Wk_u) ; v = split_heads(kv_lat @ Wv_u)
  q, k = rope(q, k)
  out = softmax(causal(q k^T / sqrt(d))) @ v   -> merge heads -> @ Wo

Sharding: 8 cores = 2 batches (data parallel) x 4-way tensor parallel over
heads (4 heads/core).  Per core:
  - q path is HOST-FUSED: Wq_eff = Wq_d @ Wq_u[:, head-slice]  (so q never
    needs the latent intermediate on-device; qT = Wq_eff^T xT directly).
  - kv latent is seq-sharded across the TP group: each core computes its
    S/4 slice of latkv, AllGathers (bf16) across the group while the q
    projection compute hides the collective, then computes k (per-head) and
    v from the gathered latents.
  - attention for its 4 heads, partial output projection over its heads'
    slice of Wo's input dim; host sums the 4 partials per batch.

On-core dataflow is transposed ([feature, seq]) so no PE transposes occur:
rotate_half is a signed-permutation matmul; softmax denominators via
ones-vector matmuls accumulated on the PE; 1/denominator applied on the
attention output.  Causal structure skips above-diagonal blocks and narrows
partial blocks.  x / projection weights / q / k / latkv run in bf16 (PSUM
accumulation fp32); attention probabilities and Wo run in f32r.
"""

import sys
from contextlib import ExitStack

sys.path.insert(0, "/opt/trn_rl_repo")

import numpy as np

import concourse.bass as bass
import concourse.mybir as mybir
import concourse.tile as tile
from concourse import bacc
from concourse.bass_utils import run_bass_kernel_spmd

HIDDEN = 2048
LATENT = 512
NUM_HEADS = 16
HEAD_DIM = 128
THETA = 10000.0
B = 2
S_FULL = 2048
N_CORES = 8
TP = 4  # tensor-parallel group size (heads 16 / 4 = 4 per core)
HPC = NUM_HEADS // TP  # heads per core
DSL = HPC * HEAD_DIM  # per-core head-dim slice width (512)

F32 = mybir.dt.float32
F32R = mybir.dt.float32r
BF16 = mybir.dt.bfloat16
FP8 = mybir.dt.float8e4
EXP_BIAS = -2.0  # keeps exp outputs under fp8e4 max; cancels in softmax

NEG = -1.0e30
SCALE = 1.0 / np.sqrt(HEAD_DIM)


def build_nc(S=S_FULL, finalize=True, iters=1, stages="full", variant="",
             gather=True):
    """Build the single-core SPMD program (same program all 8 cores).

    iters > 1 wraps the body in an on-device repeat loop; with the collective
    active (gather=True) a For_i is illegal, so repeats are unrolled instead.
    """
    nc = bacc.Bacc(None, target_bir_lowering=False)

    KC_H = HIDDEN // 128   # 16 contraction chunks for hidden dim
    KC_L = LATENT // 128   # 4 contraction chunks for latent dim
    NB = S // 512          # number of 512-wide seq blocks
    SC = S // 128          # number of 128-wide seq chunks
    SQ = S // TP           # local kv seq-shard width (512)

    xT = nc.dram_tensor("xT", [HIDDEN, S], BF16, kind="ExternalInput")
    xq = nc.dram_tensor("xq", [HIDDEN, SQ], BF16, kind="ExternalInput")
    wkvd = nc.dram_tensor("wkvd", [HIDDEN, LATENT], BF16, kind="ExternalInput")
    wqe = nc.dram_tensor("wqe", [HIDDEN, DSL], BF16, kind="ExternalInput")
    wku = nc.dram_tensor("wku", [LATENT, DSL], BF16, kind="ExternalInput")
    wvu = nc.dram_tensor("wvu", [LATENT, DSL], BF16, kind="ExternalInput")
    wo = nc.dram_tensor("wo", [DSL, HIDDEN], F32R, kind="ExternalInput")
    cosd = nc.dram_tensor("cosd", [128, S], BF16, kind="ExternalInput")
    sind = nc.dram_tensor("sind", [128, S], BF16, kind="ExternalInput")
    mtd = nc.dram_tensor("mtd", [2, 128, 256], F32, kind="ExternalInput")
    onesd = nc.dram_tensor("onesd", [128, 1], BF16, kind="ExternalInput")
    onesd8 = nc.dram_tensor("onesd8", [128, 2, 16], FP8, kind="ExternalInput")
    permd = nc.dram_tensor("permd", [128, 128], BF16, kind="ExternalInput")
    biasd = nc.dram_tensor("biasd", [128, 1], F32, kind="ExternalInput")
    MG = KC_L - 1  # latkv m-chunks exchanged via AllGather (m3 stays local)
    latpkv = nc.dram_tensor("latpkv", [128, MG, SQ], BF16)
    gkv = nc.dram_tensor("gkv", [TP, 128, MG, SQ], BF16)
    y = nc.dram_tensor("y", [S, HIDDEN], F32, kind="ExternalOutput")

    unroll = 1
    if iters < 0:
        unroll, iters = -iters, 1
    elif iters > 1 and gather:
        unroll, iters = iters, 1

    with tile.TileContext(nc) as tc, ExitStack() as _es:
        if iters > 1:
            _es.enter_context(tc.For_i(0, iters, 1))
        for _u in range(unroll):
          # ---- persistent pools (live through most of the kernel) ----
          with tc.tile_pool(name="p_qk", bufs=1) as p_qk, \
               tc.tile_pool(name="p_lat", bufs=1) as p_lat, \
               tc.tile_pool(name="p_wo", bufs=1) as p_wo, \
               tc.tile_pool(name="p_const", bufs=1) as p_const:

            qTr = p_qk.tile([128, HPC, S], BF16)        # rope'd q, [d, seq]
            kTr = p_qk.tile([128, HPC, S], BF16)        # rope'd k, [d, seq]
            latkv = p_lat.tile([128, KC_L, S], BF16)    # gathered kv latents
            wo_sb = p_wo.tile([128, HPC, HIDDEN], F32R)
            mask_sb = p_const.tile([128, 2, 256], F32)
            ones_sb = p_const.tile([128, 1], BF16)
            ones8_sb = p_const.tile([128, 2, 16], FP8)
            bias_sb = p_const.tile([128, 1], F32)
            wvu_sb = p_const.tile([128, KC_L, DSL], BF16)

            # ============ phase A+Q: kv-latent shard + direct q ============
            with tc.tile_pool(name="p_rope", bufs=1) as p_rope, \
                 tc.tile_pool(name="p_rt", bufs=3) as p_rt, \
                 tc.tile_pool(name="ps_q", bufs=3, space="PSUM") as ps_q, \
                 tc.tile_pool(name="ps_pr", bufs=2, space="PSUM") as ps_pr:
              cos_sb = p_rope.tile([128, S], BF16)
              sin_sb = p_rope.tile([128, S], BF16)
              perm_sb = p_rope.tile([128, 128], BF16)
              wqe_sb = p_rope.tile([128, KC_H, DSL], BF16)

              def rope_block(dstT, h, sl, pa, add_engine=None):
                  """dstT[:, h, sl] = cos*pa + sin*perm(pa): rotate_half as
                  a signed-permutation matmul (no PE transposes)."""
                  raw = p_rt.tile([128, 512], BF16, tag="raw")
                  nc.scalar.copy(raw, pa)
                  pr = ps_pr.tile([128, 512], F32, tag="pr")
                  nc.tensor.matmul(pr, perm_sb, raw, start=True, stop=True)
                  rawp = p_rt.tile([128, 512], BF16, tag="rawp")
                  nc.scalar.copy(rawp, pr)
                  rt = p_rt.tile([128, 512], BF16, tag="rt")
                  nc.vector.tensor_mul(dstT[:, h, sl], raw, cos_sb[:, sl])
                  nc.vector.tensor_mul(rt, rawp, sin_sb[:, sl])
                  eng = add_engine or nc.vector
                  eng.tensor_add(dstT[:, h, sl], dstT[:, h, sl], rt)

              # --- kv-local: latkv for this core's seq quarter -> DRAM ---
              wkvd3 = p_rope.tile([128, KC_H, 128], BF16)  # m3 column, Q-pass
              wku_sb = p_rope.tile([128, KC_L, DSL], BF16)
              _qes = ExitStack()
              p_xs = _qes.enter_context(tc.tile_pool(name="p_xs", bufs=2))
              with tc.tile_pool(name="p_wkvd", bufs=1) as p_wkvd, \
                   tc.tile_pool(name="p_xloc", bufs=1) as p_xloc, \
                   tc.tile_pool(name="p_st", bufs=2) as p_st, \
                   tc.tile_pool(name="ps_a", bufs=2, space="PSUM") as ps_a:
                  wkvd_sb = p_wkvd.tile([128, KC_H, MG * 128], BF16)
                  xloc = p_xloc.tile([128, KC_H, SQ], BF16)
                  # fine-grained first loads so the PE can start early
                  for kq in range(8):
                      ksl = slice(kq * 2, (kq + 1) * 2)
                      nc.sync.dma_start(
                          out=wkvd_sb[:, ksl, 0:128],
                          in_=wkvd.rearrange("(kc p) l -> p kc l", p=128)
                          [:, ksl, 0:128])
                      eng = nc.scalar if kq % 2 == 0 else nc.sync
                      eng.dma_start(
                          out=xloc[:, ksl, :],
                          in_=xq.rearrange("(kc p) s -> p kc s", p=128)
                          [:, ksl, :])
                  for m in range(1, MG):
                      nc.sync.dma_start(
                          out=wkvd_sb[:, :, m * 128:(m + 1) * 128],
                          in_=wkvd.rearrange("(kc p) l -> p kc l", p=128)
                          [:, :, m * 128:(m + 1) * 128])
                  for kq in range(4):
                      ksl = slice(kq * 4, (kq + 1) * 4)
                      eng = nc.scalar if kq % 2 == 0 else nc.sync
                      eng.dma_start(
                          out=wqe_sb[:, ksl, :],
                          in_=wqe.rearrange("(kc p) d -> p kc d", p=128)
                          [:, ksl, :])
                  nc.scalar.dma_start(
                      out=wkvd3,
                      in_=wkvd.rearrange("(kc p) l -> p kc l", p=128)
                      [:, :, MG * 128:KC_L * 128])
                  nc.scalar.dma_start(out=cos_sb, in_=cosd[:, :])
                  nc.scalar.dma_start(out=sin_sb, in_=sind[:, :])
                  nc.scalar.dma_start(out=perm_sb, in_=permd[:, :])
                  for m in range(MG):
                      acc = ps_a.tile([128, SQ], F32, tag="acc_a")
                      for kc in range(KC_H):
                          nc.tensor.matmul(
                              acc, wkvd_sb[:, kc, m * 128:(m + 1) * 128],
                              xloc[:, kc, :],
                              start=(kc == 0), stop=(kc == KC_H - 1))
                      st = p_st.tile([128, SQ], BF16, tag="st")
                      nc.vector.tensor_copy(st, acc)
                      nc.scalar.dma_start(out=latpkv[:, m, :], in_=st)

              # --- AllGather of the kv latents (overlaps the q phase) ---
              nc.gpsimd.collective_compute(
                  "AllGather", mybir.AluOpType.bypass,
                  replica_groups=[[0, 1, 2, 3], [4, 5, 6, 7]],
                  ins=[latpkv[:, :, :]], outs=[gkv[:, :, :, :]])

              # --- q: direct projection + rope, per seq block x head ---
              if True:
                  for nb in range(NB):
                      xslab = p_xs.tile([128, KC_H, 512], BF16, tag="xslab")
                      if nb == 0:
                          nc.sync.dma_start(
                              out=xslab[:, 0:8, :],
                              in_=xT.rearrange("(kc p) s -> p kc s", p=128)
                              [:, 0:8, 0:512])
                          nc.scalar.dma_start(
                              out=xslab[:, 8:16, :],
                              in_=xT.rearrange("(kc p) s -> p kc s", p=128)
                              [:, 8:16, 0:512])
                      else:
                          nc.sync.dma_start(
                              out=xslab,
                              in_=xT.rearrange("(kc p) s -> p kc s", p=128)
                              [:, :, nb * 512:(nb + 1) * 512])
                      if nb == 1:
                          nc.sync.dma_start(
                              out=wku_sb,
                              in_=wku.rearrange("(kc p) d -> p kc d", p=128))
                      if nb == 2:
                          nc.sync.dma_start(
                              out=wvu_sb,
                              in_=wvu.rearrange("(kc p) d -> p kc d", p=128))
                      for h in range(HPC):
                          pa = ps_q.tile([128, 512], F32, tag="pa")
                          for kc in range(KC_H):
                              nc.tensor.matmul(
                                  pa, wqe_sb[:, kc, h * 128:(h + 1) * 128],
                                  xslab[:, kc, :],
                                  start=(kc == 0), stop=(kc == KC_H - 1))
                          rope_block(qTr, h,
                                     slice(nb * 512, (nb + 1) * 512), pa)
                      # m3 kv-latent chunk for this slab (full S, local)
                      pm = ps_q.tile([128, 512], F32, tag="pa")
                      for kc in range(KC_H):
                          nc.tensor.matmul(
                              pm, wkvd3[:, kc, :], xslab[:, kc, :],
                              start=(kc == 0), stop=(kc == KC_H - 1))
                      nc.vector.tensor_copy(
                          latkv[:, KC_L - 1, nb * 512:(nb + 1) * 512], pm)

              _qes.close()
              # gathered latents back to SBUF, chunked per (m, t); these
              # depend on the AllGather, so issue them on the Pool queue
              # (which is busy with the collective anyway) to keep SP/ACT
              # free for the compute-feeding loads.
              for m in range(MG):
                  for t in range(TP):
                      tsl = slice(t * SQ, (t + 1) * SQ)
                      nc.gpsimd.dma_start(
                          out=latkv[:, m, tsl], in_=gkv[t, :, m, :])
              nc.gpsimd.dma_start(
                  out=mask_sb, in_=mtd.rearrange("j p c -> p j c"))
              nc.gpsimd.dma_start(out=ones_sb, in_=onesd[:, :])
              nc.gpsimd.dma_start(out=ones8_sb, in_=onesd8[:, :, :])
              nc.gpsimd.dma_start(out=bias_sb, in_=biasd[:, :])
              nc.sync.dma_start(
                  out=wo_sb, in_=wo.rearrange("(ic p) o -> p ic o", p=128))

              # --- k: up-projection from gathered latents + rope ---
              with tc.tile_pool(name="ps_k", bufs=2, space="PSUM") as ps_k:
                  for h in range(HPC):
                      for nb in range(NB):
                          pa = ps_k.tile([128, 512], F32, tag="pk")
                          for kc in range(KC_L):
                              nc.tensor.matmul(
                                  pa, wku_sb[:, kc, h * 128:(h + 1) * 128],
                                  latkv[:, kc, nb * 512:(nb + 1) * 512],
                                  start=(kc == 0), stop=(kc == KC_L - 1))
                          rope_block(kTr, h,
                                     slice(nb * 512, (nb + 1) * 512), pa)

            # ================= v for all 4 heads =================
            with tc.tile_pool(name="p_out", bufs=1) as p_out, \
                 tc.tile_pool(name="p_v", bufs=1) as p_v:
              outT = p_out.tile([128, HPC, S], F32R)  # attention out
              with tc.tile_pool(name="ps_v", bufs=4, space="PSUM") as ps_v:
                  v_sb = p_v.tile([128, SC, DSL], BF16)
                  for sc in range(SC):
                      acc = ps_v.tile([128, DSL], F32, tag="acc_v")
                      for kc in range(KC_L):
                          nc.tensor.matmul(
                              acc,
                              latkv[:, kc, sc * 128:(sc + 1) * 128],
                              wvu_sb[:, kc, :],
                              start=(kc == 0), stop=(kc == KC_L - 1))
                      if sc % 2 == 0:
                          nc.vector.tensor_copy(v_sb[:, sc, :], acc)
                      else:
                          nc.scalar.copy(v_sb[:, sc, :], acc)

              # ================= attention per head =================
              with tc.tile_pool(name="p_at", bufs=12) as p_at, \
                   tc.tile_pool(name="p_rb", bufs=2) as p_rb, \
                   tc.tile_pool(name="ps_s", bufs=2, space="PSUM") as ps_s, \
                   tc.tile_pool(name="ps_o", bufs=3, space="PSUM") as ps_o, \
                   tc.tile_pool(name="ps_n", bufs=1, space="PSUM") as ps_n:
                def emit_scores(h, qb):
                    qsl = slice(qb * 512, (qb + 1) * 512)
                    ets = []
                    for kp in range(2 * qb):
                        ps2 = ps_s.tile([128, 2, 512], F32, tag="ps")
                        for i in range(2):
                            kb = 2 * kp + i
                            nc.tensor.matmul(
                                ps2[:, i, :],
                                kTr[:, h, kb * 128:(kb + 1) * 128],
                                qTr[:, h, qsl],
                                start=True, stop=True)
                        et2 = p_at.tile([128, 2, 512], FP8, tag="et")
                        nc.scalar.activation(
                            out=et2, in_=ps2,
                            func=mybir.ActivationFunctionType.Exp,
                            scale=float(SCALE), bias=bias_sb[:, 0:1])
                        ets.append(et2)
                    etds = []
                    for j in range(4):
                        off = min(j, 2) * 128
                        w = 512 - off
                        q0 = qb * 512 + off
                        kb = 4 * qb + j
                        ps2 = ps_s.tile([128, 2, 512], F32, tag="ps")
                        nc.tensor.matmul(
                            ps2[:, 0, 0:w],
                            kTr[:, h, kb * 128:(kb + 1) * 128],
                            qTr[:, h, q0:q0 + w],
                            start=True, stop=True)
                        jj = j - off // 128
                        mw = (jj + 1) * 128
                        nc.vector.tensor_add(
                            ps2[:, 0, 0:mw], ps2[:, 0, 0:mw],
                            mask_sb[:, jj, 0:mw])
                        etd = p_at.tile([128, 512], BF16, tag="etd")
                        nc.scalar.activation(
                            out=etd[:, 0:w], in_=ps2[:, 0, 0:w],
                            func=mybir.ActivationFunctionType.Exp,
                            scale=float(SCALE), bias=bias_sb[:, 0:1])
                        etds.append(etd)
                    return h, qb, ets, etds

                def emit_av(unit):
                    h, qb, ets, etds = unit
                    hs = h * 128
                    qsl = slice(qb * 512, (qb + 1) * 512)
                    po = ps_o.tile([128, 512], F32, tag="po")
                    pn = ps_n.tile([1, 512], F32, tag="pn")
                    for kp, et2 in enumerate(ets):
                        for i in range(2):
                            kb = 2 * kp + i
                            nc.tensor.matmul(
                                po,
                                v_sb[:, kb, hs:hs + 128],
                                et2[:, i, :],
                                start=(kb == 0), stop=False)
                        nc.tensor.matmul(
                            pn[0:1, :],
                            ones8_sb[:, :, 0:1],
                            et2[:, :, :],
                            start=(kp == 0), stop=False,
                            perf_mode=mybir.MatmulPerfMode.DoubleRow)
                    for j, etd in enumerate(etds):
                        off = min(j, 2) * 128
                        w = 512 - off
                        kb = 4 * qb + j
                        nc.tensor.matmul(
                            po[:, off:512],
                            v_sb[:, kb, hs:hs + 128],
                            etd[:, 0:w],
                            start=(kb == 0), stop=(j == 3))
                        nc.tensor.matmul(
                            pn[0:1, off:512],
                            ones_sb[:, 0:1],
                            etd[:, 0:w],
                            start=(qb == 0 and j == 0), stop=(j == 3))
                    rc = p_rb.tile([1, 512], F32, tag="rc")
                    nc.vector.reciprocal(rc, pn[0:1, :])
                    rb = p_rb.tile([128, 512], F32, tag="rb")
                    nc.gpsimd.partition_broadcast(rb, rc)
                    nc.vector.tensor_mul(outT[:, h, qsl], po, rb)

                prev = None
                for h in range(HPC):
                    for qb in range(NB):
                        cur = emit_scores(h, qb)
                        if prev is not None:
                            emit_av(prev)
                        prev = cur
                emit_av(prev)

            # ================= output projection =================
              with tc.tile_pool(name="p_fin", bufs=3) as p_fin, \
                   tc.tile_pool(name="ps_d", bufs=4, space="PSUM") as ps_d:
                for sc in range(SC):
                    fin = p_fin.tile([128, HIDDEN], F32, tag="fin")
                    for ob in range(HIDDEN // 512):
                        acc = ps_d.tile([128, 512], F32, tag="acc_d")
                        for ic in range(HPC):
                            nc.tensor.matmul(
                                acc,
                                outT[:, ic, sc * 128:(sc + 1) * 128],
                                wo_sb[:, ic, ob * 512:(ob + 1) * 512],
                                start=(ic == 0), stop=(ic == HPC - 1))
                        osl = slice(ob * 512, (ob + 1) * 512)
                        if ob % 2 == 0:
                            nc.scalar.copy(fin[:, osl], acc)
                        else:
                            nc.vector.tensor_copy(fin[:, osl], acc)
                    if sc == SC - 1:
                        for oq in range(4):
                            osl4 = slice(oq * 512, (oq + 1) * 512)
                            eng = nc.sync if oq % 2 == 0 else nc.scalar
                            eng.dma_start(
                                out=y[sc * 128:(sc + 1) * 128, osl4],
                                in_=fin[:, osl4])
                    else:
                        nc.sync.dma_start(
                            out=y[sc * 128:(sc + 1) * 128, :], in_=fin)

    if finalize:
        nc.finalize()
    return nc


# ---------------------------------------------------------------------------
# host-side helpers


def host_inputs(x, Wq_d, Wkv_d, Wq_u, Wk_u, Wv_u, Wo, S=S_FULL, gather=True):
    """Build the 8 per-core input maps from full inputs."""
    import ml_dtypes
    bf16 = ml_dtypes.bfloat16

    x = np.asarray(x, dtype=np.float32)
    Wq_d = np.asarray(Wq_d, dtype=np.float64)
    Wkv_d = np.asarray(Wkv_d, dtype=np.float32)
    Wq_u = np.asarray(Wq_u, dtype=np.float64)
    Wk_u = np.asarray(Wk_u, dtype=np.float32)
    Wv_u = np.asarray(Wv_u, dtype=np.float32)
    Wo = np.asarray(Wo, dtype=np.float32)

    inv_freq = 1.0 / (THETA ** (np.arange(0, HEAD_DIM, 2, dtype=np.float64)
                                / HEAD_DIM))  # (64,)
    pos = np.arange(S, dtype=np.float64)
    ang = pos[None, :] * np.concatenate([inv_freq, inv_freq])[:, None]  # (128, S)
    COS = np.cos(ang).astype(bf16)
    SIN = np.sin(ang).astype(bf16)

    # signed permutation for rotate_half in [d, seq] layout:
    # out[m] = -in[m+64] for m<64 ; +in[m-64] for m>=64
    PERM = np.zeros((128, 128), dtype=np.float32)
    for m in range(64):
        PERM[m + 64, m] = -1.0
        PERM[m, m + 64] = 1.0

    # masks for narrowed diagonal blocks: jj=0 -> mask c<r on first 128 cols;
    # jj=1 -> mask c<128+r on first 256 cols
    MT = np.zeros((2, 128, 256), dtype=np.float32)
    r = np.arange(128)[:, None]
    c = np.arange(256)[None, :]
    MT[0] = np.where(c >= r, 0.0, NEG)
    MT[1] = np.where(c >= 128 + r, 0.0, NEG)

    in_maps = []
    for core in range(N_CORES):
        b, tp = core // TP, core % TP
        sl = slice(tp * DSL, (tp + 1) * DSL)
        SQ = S // TP
        xt_c = np.ascontiguousarray(x[b, :S].T.astype(bf16))
        wqe = (Wq_d @ Wq_u[:, sl]).astype(bf16)
        in_maps.append({
            "xT": xt_c,
            "xq": np.ascontiguousarray(xt_c[:, tp * SQ:(tp + 1) * SQ]),
            "wkvd": Wkv_d.astype(bf16),
            "wqe": np.ascontiguousarray(wqe),
            "wku": np.ascontiguousarray(Wk_u[:, sl].astype(bf16)),
            "wvu": np.ascontiguousarray(Wv_u[:, sl].astype(bf16)),
            "wo": np.ascontiguousarray(Wo[sl, :]),
            "cosd": COS,
            "sind": SIN,
            "mtd": MT,
            "onesd": np.ones((128, 1), dtype=ml_dtypes.bfloat16),
            "onesd8": np.ones((128, 2, 16), dtype=ml_dtypes.float8_e4m3fn),
            "permd": PERM.astype(bf16),
            "biasd": np.full((128, 1), EXP_BIAS, dtype=np.float32),
        })
    return in_maps


def assemble(results, S=S_FULL):
    out = np.zeros((B, S, HIDDEN), dtype=np.float32)
    for core in range(N_CORES):
        out[core // TP] += results[core]["y"]
    return out


_NC_CACHE = {}


def kernel(x, Wq_d, Wkv_d, Wq_u, Wk_u, Wv_u, Wo):
    S = x.shape[1]
    if S not in _NC_CACHE:
        _NC_CACHE[S] = build_nc(S)
    nc = _NC_CACHE[S]
    in_maps = host_inputs(x, Wq_d, Wkv_d, Wq_u, Wk_u, Wv_u, Wo, S=S)

    res = run_bass_kernel_spmd(nc, in_maps, list(range(N_CORES)))
    return assemble(res.results, S=S)


# revision 51
# speedup vs baseline: 1.3145x; 1.2599x over previous
"""Trainium2 Bass kernel for MultiHeadLatentAttention.

Reference computation (B=2, S=2048, HIDDEN=2048, 16 heads x 128, LATENT=512):
  q_lat = x @ Wq_d ; kv_lat = x @ Wkv_d
  q = split_heads(q_lat @ Wq_u) ; k = split_heads(kv_lat @ Wk_u) ; v = split_heads(kv_lat @ Wv_u)
  q, k = rope(q, k)
  out = softmax(causal(q k^T / sqrt(d))) @ v   -> merge heads -> @ Wo

Sharding: 8 cores = 2 batches (data parallel) x 4-way tensor parallel over
heads (4 heads/core).  Per core:
  - q path is HOST-FUSED: Wq_eff = Wq_d @ Wq_u[:, head-slice]  (so q never
    needs the latent intermediate on-device; qT = Wq_eff^T xT directly).
  - kv latent is seq-sharded across the TP group: each core computes its
    S/4 slice of latkv, AllGathers (bf16) across the group while the q
    projection compute hides the collective, then computes k (per-head) and
    v from the gathered latents.
  - attention for its 4 heads, partial output projection over its heads'
    slice of Wo's input dim; host sums the 4 partials per batch.

On-core dataflow is transposed ([feature, seq]) so no PE transposes occur:
rotate_half is a signed-permutation matmul; softmax denominators via
ones-vector matmuls accumulated on the PE; 1/denominator applied on the
attention output.  Causal structure skips above-diagonal blocks and narrows
partial blocks.  x / projection weights / q / k / latkv run in bf16 (PSUM
accumulation fp32); attention probabilities and Wo run in f32r.
"""

import sys
from contextlib import ExitStack

sys.path.insert(0, "/opt/trn_rl_repo")

import numpy as np

import concourse.bass as bass
import concourse.mybir as mybir
import concourse.tile as tile
from concourse import bacc
from concourse.bass_utils import run_bass_kernel_spmd

HIDDEN = 2048
LATENT = 512
NUM_HEADS = 16
HEAD_DIM = 128
THETA = 10000.0
B = 2
S_FULL = 2048
N_CORES = 8
TP = 4  # tensor-parallel group size (heads 16 / 4 = 4 per core)
HPC = NUM_HEADS // TP  # heads per core
DSL = HPC * HEAD_DIM  # per-core head-dim slice width (512)

F32 = mybir.dt.float32
F32R = mybir.dt.float32r
BF16 = mybir.dt.bfloat16
FP8 = mybir.dt.float8e4
EXP_BIAS = -2.0  # keeps exp outputs under fp8e4 max; cancels in softmax

NEG = -1.0e30
SCALE = 1.0 / np.sqrt(HEAD_DIM)


def build_nc(S=S_FULL, finalize=True, iters=1, stages="full", variant="",
             gather=True):
    """Build the single-core SPMD program (same program all 8 cores).

    iters > 1 wraps the body in an on-device repeat loop; with the collective
    active (gather=True) a For_i is illegal, so repeats are unrolled instead.
    """
    nc = bacc.Bacc(None, target_bir_lowering=False)

    KC_H = HIDDEN // 128   # 16 contraction chunks for hidden dim
    KC_L = LATENT // 128   # 4 contraction chunks for latent dim
    NB = S // 512          # number of 512-wide seq blocks
    SC = S // 128          # number of 128-wide seq chunks
    SQ = S // TP           # local kv seq-shard width (512)

    xT = nc.dram_tensor("xT", [HIDDEN, S], BF16, kind="ExternalInput")
    xq = nc.dram_tensor("xq", [HIDDEN, SQ], BF16, kind="ExternalInput")
    wkvd = nc.dram_tensor("wkvd", [HIDDEN, LATENT], BF16, kind="ExternalInput")
    wqe = nc.dram_tensor("wqe", [HIDDEN, DSL], BF16, kind="ExternalInput")
    wku = nc.dram_tensor("wku", [LATENT, DSL], BF16, kind="ExternalInput")
    wvu = nc.dram_tensor("wvu", [LATENT, DSL], BF16, kind="ExternalInput")
    wo = nc.dram_tensor("wo", [DSL, HIDDEN], F32R, kind="ExternalInput")
    cosd = nc.dram_tensor("cosd", [128, S], BF16, kind="ExternalInput")
    sind = nc.dram_tensor("sind", [128, S], BF16, kind="ExternalInput")
    mtd = nc.dram_tensor("mtd", [2, 128, 256], F32, kind="ExternalInput")
    onesd = nc.dram_tensor("onesd", [128, 1], BF16, kind="ExternalInput")
    onesd8 = nc.dram_tensor("onesd8", [128, 2, 16], FP8, kind="ExternalInput")
    permd = nc.dram_tensor("permd", [128, 128], BF16, kind="ExternalInput")
    biasd = nc.dram_tensor("biasd", [128, 1], F32, kind="ExternalInput")
    MG = KC_L - 1  # latkv m-chunks exchanged via AllGather (m3 stays local)
    latpkv = nc.dram_tensor("latpkv", [128, MG, SQ], BF16)
    gkv = nc.dram_tensor("gkv", [TP, 128, MG, SQ], BF16)
    y = nc.dram_tensor("y", [S, HIDDEN], F32, kind="ExternalOutput")

    unroll = 1
    if iters < 0:
        unroll, iters = -iters, 1
    elif iters > 1 and gather:
        unroll, iters = iters, 1

    with tile.TileContext(nc) as tc, ExitStack() as _es:
        if iters > 1:
            _es.enter_context(tc.For_i(0, iters, 1))
        for _u in range(unroll):
          # ---- persistent pools (live through most of the kernel) ----
          with tc.tile_pool(name="p_qk", bufs=1) as p_qk, \
               tc.tile_pool(name="p_lat", bufs=1) as p_lat, \
               tc.tile_pool(name="p_wo", bufs=1) as p_wo, \
               tc.tile_pool(name="p_const", bufs=1) as p_const:

            qTr = p_qk.tile([128, HPC, S], BF16)        # rope'd q, [d, seq]
            kTr = p_qk.tile([128, HPC, S], BF16)        # rope'd k, [d, seq]
            latkv = p_lat.tile([128, KC_L, S], BF16)    # gathered kv latents
            v_sb = p_lat.tile([128, SC, DSL], BF16)     # v, [seq-chunk, d]
            wo_sb = p_wo.tile([128, HPC, HIDDEN], F32R)
            mask_sb = p_const.tile([128, 2, 256], F32)
            ones_sb = p_const.tile([128, 1], BF16)
            ones8_sb = p_const.tile([128, 2, 16], FP8)
            bias_sb = p_const.tile([128, 1], F32)
            wvu_sb = p_const.tile([128, KC_L, DSL], BF16)

            # ============ phase A+Q: kv-latent shard + direct q ============
            with tc.tile_pool(name="p_rope", bufs=1) as p_rope, \
                 tc.tile_pool(name="p_rt", bufs=3) as p_rt, \
                 tc.tile_pool(name="ps_q", bufs=2, space="PSUM") as ps_q, \
                 tc.tile_pool(name="ps_pr", bufs=2, space="PSUM") as ps_pr:
              cos_sb = p_rope.tile([128, S], BF16)
              sin_sb = p_rope.tile([128, S], BF16)
              perm_sb = p_rope.tile([128, 128], BF16)
              wqe_sb = p_rope.tile([128, KC_H, DSL], BF16)

              def rope_block(dstT, h, sl, pa, add_engine=None):
                  """dstT[:, h, sl] = cos*pa + sin*perm(pa): rotate_half as
                  a signed-permutation matmul (no PE transposes)."""
                  raw = p_rt.tile([128, 512], BF16, tag="raw")
                  nc.scalar.copy(raw, pa)
                  pr = ps_pr.tile([128, 512], F32, tag="pr")
                  nc.tensor.matmul(pr, perm_sb, raw, start=True, stop=True)
                  rawp = p_rt.tile([128, 512], BF16, tag="rawp")
                  nc.scalar.copy(rawp, pr)
                  rt = p_rt.tile([128, 512], BF16, tag="rt")
                  nc.vector.tensor_mul(dstT[:, h, sl], raw, cos_sb[:, sl])
                  nc.vector.tensor_mul(rt, rawp, sin_sb[:, sl])
                  eng = add_engine or nc.vector
                  eng.tensor_add(dstT[:, h, sl], dstT[:, h, sl], rt)

              # --- kv-local: latkv for this core's seq quarter -> DRAM ---
              wkvd3 = p_rope.tile([128, KC_H, 128], BF16)  # m3 column, Q-pass
              wku_sb = p_rope.tile([128, KC_L, DSL], BF16)
              _qes = ExitStack()
              p_xs = _qes.enter_context(tc.tile_pool(name="p_xs", bufs=2))
              with tc.tile_pool(name="p_wkvd", bufs=1) as p_wkvd, \
                   tc.tile_pool(name="p_xloc", bufs=1) as p_xloc, \
                   tc.tile_pool(name="p_st", bufs=2) as p_st, \
                   tc.tile_pool(name="ps_a", bufs=2, space="PSUM") as ps_a:
                  wkvd_sb = p_wkvd.tile([128, KC_H, MG * 128], BF16)
                  xloc = p_xloc.tile([128, KC_H, SQ], BF16)
                  # fine-grained first loads so the PE can start early
                  for kq in range(8):
                      ksl = slice(kq * 2, (kq + 1) * 2)
                      nc.sync.dma_start(
                          out=wkvd_sb[:, ksl, 0:128],
                          in_=wkvd.rearrange("(kc p) l -> p kc l", p=128)
                          [:, ksl, 0:128])
                      nc.scalar.dma_start(
                          out=xloc[:, ksl, :],
                          in_=xq.rearrange("(kc p) s -> p kc s", p=128)
                          [:, ksl, :])
                  for m in range(1, MG):
                      nc.sync.dma_start(
                          out=wkvd_sb[:, :, m * 128:(m + 1) * 128],
                          in_=wkvd.rearrange("(kc p) l -> p kc l", p=128)
                          [:, :, m * 128:(m + 1) * 128])
                  for kq in range(4):
                      ksl = slice(kq * 4, (kq + 1) * 4)
                      nc.scalar.dma_start(
                          out=wqe_sb[:, ksl, :],
                          in_=wqe.rearrange("(kc p) d -> p kc d", p=128)
                          [:, ksl, :])
                  nc.scalar.dma_start(
                      out=wkvd3,
                      in_=wkvd.rearrange("(kc p) l -> p kc l", p=128)
                      [:, :, MG * 128:KC_L * 128])
                  nc.scalar.dma_start(out=cos_sb, in_=cosd[:, :])
                  nc.scalar.dma_start(out=sin_sb, in_=sind[:, :])
                  nc.scalar.dma_start(out=perm_sb, in_=permd[:, :])
                  for m in range(MG):
                      acc = ps_a.tile([128, SQ], F32, tag="acc_a")
                      for kc in range(KC_H):
                          nc.tensor.matmul(
                              acc, wkvd_sb[:, kc, m * 128:(m + 1) * 128],
                              xloc[:, kc, :],
                              start=(kc == 0), stop=(kc == KC_H - 1))
                      st = p_st.tile([128, SQ], BF16, tag="st")
                      nc.vector.tensor_copy(st, acc)
                      nc.scalar.dma_start(out=latpkv[:, m, :], in_=st)

              # --- AllGather of the kv latents (overlaps the q phase) ---
              nc.gpsimd.collective_compute(
                  "AllGather", mybir.AluOpType.bypass,
                  replica_groups=[[0, 1, 2, 3], [4, 5, 6, 7]],
                  ins=[latpkv[:, :, :]], outs=[gkv[:, :, :, :]])

              # --- q: direct projection + rope, per seq block x head ---
              if True:
                  for nb in range(NB):
                      xslab = p_xs.tile([128, KC_H, 512], BF16, tag="xslab")
                      nc.sync.dma_start(
                          out=xslab,
                          in_=xT.rearrange("(kc p) s -> p kc s", p=128)
                          [:, :, nb * 512:(nb + 1) * 512])
                      if nb == 1:
                          nc.sync.dma_start(
                              out=wku_sb,
                              in_=wku.rearrange("(kc p) d -> p kc d", p=128))
                      if nb == 2:
                          nc.sync.dma_start(
                              out=wvu_sb,
                              in_=wvu.rearrange("(kc p) d -> p kc d", p=128))
                      for h in range(HPC):
                          pa = ps_q.tile([128, 512], F32, tag="pa")
                          for kc in range(KC_H):
                              nc.tensor.matmul(
                                  pa, wqe_sb[:, kc, h * 128:(h + 1) * 128],
                                  xslab[:, kc, :],
                                  start=(kc == 0), stop=(kc == KC_H - 1))
                          rope_block(qTr, h,
                                     slice(nb * 512, (nb + 1) * 512), pa)
                      # m3 kv-latent chunk for this slab (full S, local)
                      pm = ps_q.tile([128, 512], F32, tag="pa")
                      for kc in range(KC_H):
                          nc.tensor.matmul(
                              pm, wkvd3[:, kc, :], xslab[:, kc, :],
                              start=(kc == 0), stop=(kc == KC_H - 1))
                      nc.vector.tensor_copy(
                          latkv[:, KC_L - 1, nb * 512:(nb + 1) * 512], pm)

              _qes.close()
              # gathered latents back to SBUF, chunked per (m, t); these
              # depend on the AllGather, so issue them on the Pool queue
              # (which is busy with the collective anyway) to keep SP/ACT
              # free for the compute-feeding loads.
              for m in range(MG):
                  for t in range(TP):
                      tsl = slice(t * SQ, (t + 1) * SQ)
                      nc.gpsimd.dma_start(
                          out=latkv[:, m, tsl], in_=gkv[t, :, m, :])
              nc.gpsimd.dma_start(
                  out=mask_sb, in_=mtd.rearrange("j p c -> p j c"))
              nc.gpsimd.dma_start(out=ones_sb, in_=onesd[:, :])
              nc.gpsimd.dma_start(out=ones8_sb, in_=onesd8[:, :, :])
              nc.gpsimd.dma_start(out=bias_sb, in_=biasd[:, :])
              nc.sync.dma_start(
                  out=wo_sb, in_=wo.rearrange("(ic p) o -> p ic o", p=128))


              # --- k up-proj + rope interleaved with v chains ---
              with tc.tile_pool(name="ps_v", bufs=2, space="PSUM") as ps_v, \
                   tc.tile_pool(name="ps_k", bufs=2, space="PSUM") as ps_k:
                  for h in range(HPC):
                      for nb in range(NB):
                          pa = ps_k.tile([128, 512], F32, tag="pk")
                          for kc in range(KC_L):
                              nc.tensor.matmul(
                                  pa, wku_sb[:, kc, h * 128:(h + 1) * 128],
                                  latkv[:, kc, nb * 512:(nb + 1) * 512],
                                  start=(kc == 0), stop=(kc == KC_L - 1))
                          rope_block(kTr, h,
                                     slice(nb * 512, (nb + 1) * 512), pa)
                          sc = h * 4 + nb
                          acc = ps_v.tile([128, DSL], F32, tag="acc_v")
                          for kc in range(KC_L):
                              nc.tensor.matmul(
                                  acc,
                                  latkv[:, kc, sc * 128:(sc + 1) * 128],
                                  wvu_sb[:, kc, :],
                                  start=(kc == 0), stop=(kc == KC_L - 1))
                          if sc % 2 == 0:
                              nc.vector.tensor_copy(v_sb[:, sc, :], acc)
                          else:
                              nc.scalar.copy(v_sb[:, sc, :], acc)

            # ================= attention =================
            with tc.tile_pool(name="p_out", bufs=1) as p_out:
              outT = p_out.tile([128, HPC, S], F32R)  # attention out
              # ================= attention per head =================
              with tc.tile_pool(name="p_at", bufs=12) as p_at, \
                   tc.tile_pool(name="p_rb", bufs=2) as p_rb, \
                   tc.tile_pool(name="ps_s", bufs=2, space="PSUM") as ps_s, \
                   tc.tile_pool(name="ps_o", bufs=3, space="PSUM") as ps_o, \
                   tc.tile_pool(name="ps_n", bufs=1, space="PSUM") as ps_n:
                def emit_scores(h, qb):
                    qsl = slice(qb * 512, (qb + 1) * 512)
                    ets = []
                    for kp in range(2 * qb):
                        ps2 = ps_s.tile([128, 2, 512], F32, tag="ps")
                        for i in range(2):
                            kb = 2 * kp + i
                            nc.tensor.matmul(
                                ps2[:, i, :],
                                kTr[:, h, kb * 128:(kb + 1) * 128],
                                qTr[:, h, qsl],
                                start=True, stop=True)
                        et2 = p_at.tile([128, 2, 512], FP8, tag="et")
                        nc.scalar.activation(
                            out=et2, in_=ps2,
                            func=mybir.ActivationFunctionType.Exp,
                            scale=float(SCALE), bias=bias_sb[:, 0:1])
                        ets.append(et2)
                    etds = []
                    for j in range(4):
                        off = min(j, 2) * 128
                        w = 512 - off
                        q0 = qb * 512 + off
                        kb = 4 * qb + j
                        ps2 = ps_s.tile([128, 2, 512], F32, tag="ps")
                        nc.tensor.matmul(
                            ps2[:, 0, 0:w],
                            kTr[:, h, kb * 128:(kb + 1) * 128],
                            qTr[:, h, q0:q0 + w],
                            start=True, stop=True)
                        jj = j - off // 128
                        mw = (jj + 1) * 128
                        nc.vector.tensor_add(
                            ps2[:, 0, 0:mw], ps2[:, 0, 0:mw],
                            mask_sb[:, jj, 0:mw])
                        etd = p_at.tile([128, 512], BF16, tag="etd")
                        nc.scalar.activation(
                            out=etd[:, 0:w], in_=ps2[:, 0, 0:w],
                            func=mybir.ActivationFunctionType.Exp,
                            scale=float(SCALE), bias=bias_sb[:, 0:1])
                        etds.append(etd)
                    return h, qb, ets, etds

                def emit_av(unit):
                    h, qb, ets, etds = unit
                    hs = h * 128
                    qsl = slice(qb * 512, (qb + 1) * 512)
                    po = ps_o.tile([128, 512], F32, tag="po")
                    pn = ps_n.tile([1, 512], F32, tag="pn")
                    for kp, et2 in enumerate(ets):
                        for i in range(2):
                            kb = 2 * kp + i
                            nc.tensor.matmul(
                                po,
                                v_sb[:, kb, hs:hs + 128],
                                et2[:, i, :],
                                start=(kb == 0), stop=False)
                        nc.tensor.matmul(
                            pn[0:1, :],
                            ones8_sb[:, :, 0:1],
                            et2[:, :, :],
                            start=(kp == 0), stop=False,
                            perf_mode=mybir.MatmulPerfMode.DoubleRow)
                    for j, etd in enumerate(etds):
                        off = min(j, 2) * 128
                        w = 512 - off
                        kb = 4 * qb + j
                        nc.tensor.matmul(
                            po[:, off:512],
                            v_sb[:, kb, hs:hs + 128],
                            etd[:, 0:w],
                            start=(kb == 0), stop=(j == 3))
                        nc.tensor.matmul(
                            pn[0:1, off:512],
                            ones_sb[:, 0:1],
                            etd[:, 0:w],
                            start=(qb == 0 and j == 0), stop=(j == 3))
                    rc = p_rb.tile([1, 512], F32, tag="rc")
                    nc.vector.reciprocal(rc, pn[0:1, :])
                    rb = p_rb.tile([128, 512], F32, tag="rb")
                    nc.gpsimd.partition_broadcast(rb, rc)
                    nc.vector.tensor_mul(outT[:, h, qsl], po, rb)

                prev = None
                for h in range(HPC):
                    for qb in range(NB):
                        cur = emit_scores(h, qb)
                        if prev is not None:
                            emit_av(prev)
                        prev = cur
                emit_av(prev)

            # ================= output projection =================
              with tc.tile_pool(name="p_fin", bufs=3) as p_fin, \
                   tc.tile_pool(name="ps_d", bufs=4, space="PSUM") as ps_d:
                for sc in range(SC):
                    fin = p_fin.tile([128, HIDDEN], F32, tag="fin")
                    for ob in range(HIDDEN // 512):
                        acc = ps_d.tile([128, 512], F32, tag="acc_d")
                        for ic in range(HPC):
                            nc.tensor.matmul(
                                acc,
                                outT[:, ic, sc * 128:(sc + 1) * 128],
                                wo_sb[:, ic, ob * 512:(ob + 1) * 512],
                                start=(ic == 0), stop=(ic == HPC - 1))
                        osl = slice(ob * 512, (ob + 1) * 512)
                        if ob % 2 == 0:
                            nc.scalar.copy(fin[:, osl], acc)
                        else:
                            nc.vector.tensor_copy(fin[:, osl], acc)
                    if sc == SC - 1:
                        for oq in range(4):
                            osl4 = slice(oq * 512, (oq + 1) * 512)
                            nc.sync.dma_start(
                                out=y[sc * 128:(sc + 1) * 128, osl4],
                                in_=fin[:, osl4])
                    else:
                        nc.sync.dma_start(
                            out=y[sc * 128:(sc + 1) * 128, :], in_=fin)

    if finalize:
        nc.finalize()
    return nc


# ---------------------------------------------------------------------------
# host-side helpers


def host_inputs(x, Wq_d, Wkv_d, Wq_u, Wk_u, Wv_u, Wo, S=S_FULL, gather=True):
    """Build the 8 per-core input maps from full inputs."""
    import ml_dtypes
    bf16 = ml_dtypes.bfloat16

    x = np.asarray(x, dtype=np.float32)
    Wq_d = np.asarray(Wq_d, dtype=np.float64)
    Wkv_d = np.asarray(Wkv_d, dtype=np.float32)
    Wq_u = np.asarray(Wq_u, dtype=np.float64)
    Wk_u = np.asarray(Wk_u, dtype=np.float32)
    Wv_u = np.asarray(Wv_u, dtype=np.float32)
    Wo = np.asarray(Wo, dtype=np.float32)

    inv_freq = 1.0 / (THETA ** (np.arange(0, HEAD_DIM, 2, dtype=np.float64)
                                / HEAD_DIM))  # (64,)
    pos = np.arange(S, dtype=np.float64)
    ang = pos[None, :] * np.concatenate([inv_freq, inv_freq])[:, None]  # (128, S)
    COS = np.cos(ang).astype(bf16)
    SIN = np.sin(ang).astype(bf16)

    # signed permutation for rotate_half in [d, seq] layout:
    # out[m] = -in[m+64] for m<64 ; +in[m-64] for m>=64
    PERM = np.zeros((128, 128), dtype=np.float32)
    for m in range(64):
        PERM[m + 64, m] = -1.0
        PERM[m, m + 64] = 1.0

    # masks for narrowed diagonal blocks: jj=0 -> mask c<r on first 128 cols;
    # jj=1 -> mask c<128+r on first 256 cols
    MT = np.zeros((2, 128, 256), dtype=np.float32)
    r = np.arange(128)[:, None]
    c = np.arange(256)[None, :]
    MT[0] = np.where(c >= r, 0.0, NEG)
    MT[1] = np.where(c >= 128 + r, 0.0, NEG)

    in_maps = []
    for core in range(N_CORES):
        b, tp = core // TP, core % TP
        sl = slice(tp * DSL, (tp + 1) * DSL)
        SQ = S // TP
        xt_c = np.ascontiguousarray(x[b, :S].T.astype(bf16))
        wqe = (Wq_d @ Wq_u[:, sl]).astype(bf16)
        in_maps.append({
            "xT": xt_c,
            "xq": np.ascontiguousarray(xt_c[:, tp * SQ:(tp + 1) * SQ]),
            "wkvd": Wkv_d.astype(bf16),
            "wqe": np.ascontiguousarray(wqe),
            "wku": np.ascontiguousarray(Wk_u[:, sl].astype(bf16)),
            "wvu": np.ascontiguousarray(Wv_u[:, sl].astype(bf16)),
            "wo": np.ascontiguousarray(Wo[sl, :]),
            "cosd": COS,
            "sind": SIN,
            "mtd": MT,
            "onesd": np.ones((128, 1), dtype=ml_dtypes.bfloat16),
            "onesd8": np.ones((128, 2, 16), dtype=ml_dtypes.float8_e4m3fn),
            "permd": PERM.astype(bf16),
            "biasd": np.full((128, 1), EXP_BIAS, dtype=np.float32),
        })
    return in_maps


def assemble(results, S=S_FULL):
    out = np.zeros((B, S, HIDDEN), dtype=np.float32)
    for core in range(N_CORES):
        out[core // TP] += results[core]["y"]
    return out


_NC_CACHE = {}


def kernel(x, Wq_d, Wkv_d, Wq_u, Wk_u, Wv_u, Wo):
    S = x.shape[1]
    if S not in _NC_CACHE:
        _NC_CACHE[S] = build_nc(S)
    nc = _NC_CACHE[S]
    in_maps = host_inputs(x, Wq_d, Wkv_d, Wq_u, Wk_u, Wv_u, Wo, S=S)

    res = run_bass_kernel_spmd(nc, in_maps, list(range(N_CORES)))
    return assemble(res.results, S=S)


# revision 52
# speedup vs baseline: 1.4296x; 1.0875x over previous
"""Trainium2 Bass kernel for MultiHeadLatentAttention.

Reference computation (B=2, S=2048, HIDDEN=2048, 16 heads x 128, LATENT=512):
  q_lat = x @ Wq_d ; kv_lat = x @ Wkv_d
  q = split_heads(q_lat @ Wq_u) ; k = split_heads(kv_lat @ Wk_u) ; v = split_heads(kv_lat @ Wv_u)
  q, k = rope(q, k)
  out = softmax(causal(q k^T / sqrt(d))) @ v   -> merge heads -> @ Wo

Sharding: 8 cores = 2 batches (data parallel) x 4-way tensor parallel over
heads (4 heads/core).  Per core:
  - q path is HOST-FUSED: Wq_eff = Wq_d @ Wq_u[:, head-slice], so q is
    projected straight from x with no latent intermediate on-device.
  - kv latents m0-m2 are seq-sharded across the TP group and AllGathered
    (bf16, ~1.5MB) while the q projection hides the collective; the m3
    chunk is computed locally for the full sequence inside the q pass.
  - k (per-head) and v come from the gathered latents; attention runs for
    the core's 4 heads; the partial output projection covers its heads'
    slice of Wo's input dim and the host sums 4 partials per batch.

On-core dataflow is transposed ([feature, seq]) so no PE transposes occur;
rotate_half is a signed-permutation matmul.  Inputs/projections run in bf16
(fp32 PSUM accumulation).  Attention: scores bf16, probabilities stored in
fp8e4 for full off-diagonal blocks (exp biased by -2 so the fp8 range
holds; the bias cancels in softmax) which lets the softmax denominators
use fp8 DoubleRow matmuls (half rate per pair); diagonal blocks keep bf16
probabilities so short-context rows cannot underflow to a zero denominator.
Causal structure skips above-diagonal blocks and narrows partial blocks.
The attention loop is software-pipelined at (head, q-block) granularity:
unit k+1's scores+exp stream while unit k's AV/denominator chains run.

DMA queues: SP carries the PE-feeding loads, the ACT engine queue carries
secondary loads/stores, and collective-dependent transfers ride the Pool
queue behind the AllGather so they never block the other queues.
"""

import sys
from contextlib import ExitStack

sys.path.insert(0, "/opt/trn_rl_repo")

import numpy as np

import concourse.bass as bass
import concourse.mybir as mybir
import concourse.tile as tile
from concourse import bacc
from concourse.bass_utils import run_bass_kernel_spmd

HIDDEN = 2048
LATENT = 512
NUM_HEADS = 16
HEAD_DIM = 128
THETA = 10000.0
B = 2
S_FULL = 2048
N_CORES = 8
TP = 4  # tensor-parallel group size (heads 16 / 4 = 4 per core)
HPC = NUM_HEADS // TP  # heads per core
DSL = HPC * HEAD_DIM  # per-core head-dim slice width (512)

F32 = mybir.dt.float32
F32R = mybir.dt.float32r
BF16 = mybir.dt.bfloat16
FP8 = mybir.dt.float8e4
EXP_BIAS = -2.0  # keeps exp outputs under fp8e4 max; cancels in softmax

NEG = -1.0e30
SCALE = 1.0 / np.sqrt(HEAD_DIM)


def build_nc(S=S_FULL, finalize=True, iters=1, gather=True):
    """Build the single-core SPMD program (same program all 8 cores).

    iters > 1 wraps the body in an on-device repeat loop; with the collective
    active (gather=True) a For_i is illegal, so repeats are unrolled instead.
    """
    nc = bacc.Bacc(None, target_bir_lowering=False)

    KC_H = HIDDEN // 128   # 16 contraction chunks for hidden dim
    KC_L = LATENT // 128   # 4 contraction chunks for latent dim
    NB = S // 512          # number of 512-wide seq blocks
    SC = S // 128          # number of 128-wide seq chunks
    SQ = S // TP           # local kv seq-shard width (512)

    xT = nc.dram_tensor("xT", [HIDDEN, S], BF16, kind="ExternalInput")
    xq = nc.dram_tensor("xq", [HIDDEN, SQ], BF16, kind="ExternalInput")
    wkvd = nc.dram_tensor("wkvd", [HIDDEN, LATENT], BF16, kind="ExternalInput")
    wqe = nc.dram_tensor("wqe", [HIDDEN, DSL], BF16, kind="ExternalInput")
    wku = nc.dram_tensor("wku", [LATENT, DSL], BF16, kind="ExternalInput")
    wvu = nc.dram_tensor("wvu", [LATENT, DSL], BF16, kind="ExternalInput")
    wo = nc.dram_tensor("wo", [DSL, HIDDEN], F32R, kind="ExternalInput")
    cosd = nc.dram_tensor("cosd", [128, S], BF16, kind="ExternalInput")
    sind = nc.dram_tensor("sind", [128, S], BF16, kind="ExternalInput")
    mtd = nc.dram_tensor("mtd", [2, 128, 256], F32, kind="ExternalInput")
    onesd = nc.dram_tensor("onesd", [128, 1], BF16, kind="ExternalInput")
    onesd8 = nc.dram_tensor("onesd8", [128, 2, 16], FP8, kind="ExternalInput")
    permd = nc.dram_tensor("permd", [128, 128], BF16, kind="ExternalInput")
    biasd = nc.dram_tensor("biasd", [128, 1], F32, kind="ExternalInput")
    MG = KC_L - 1  # latkv m-chunks exchanged via AllGather (m3 stays local)
    latpkv = nc.dram_tensor("latpkv", [128, MG, SQ], BF16)
    gkv = nc.dram_tensor("gkv", [TP, 128, MG, SQ], BF16)
    y = nc.dram_tensor("y", [S, HIDDEN], F32, kind="ExternalOutput")

    unroll = 1
    if iters < 0:
        unroll, iters = -iters, 1
    elif iters > 1 and gather:
        unroll, iters = iters, 1

    with tile.TileContext(nc) as tc, ExitStack() as _es:
        if iters > 1:
            _es.enter_context(tc.For_i(0, iters, 1))
        for _u in range(unroll):
          # ---- persistent pools (live through most of the kernel) ----
          with tc.tile_pool(name="p_qk", bufs=1) as p_qk, \
               tc.tile_pool(name="p_lat", bufs=1) as p_lat, \
               tc.tile_pool(name="p_wo", bufs=1) as p_wo, \
               tc.tile_pool(name="p_const", bufs=1) as p_const:

            qTr = p_qk.tile([128, HPC, S], BF16)        # rope'd q, [d, seq]
            kTr = p_qk.tile([128, HPC, S], BF16)        # rope'd k, [d, seq]
            latkv = p_lat.tile([128, KC_L, S], BF16)    # gathered kv latents
            v_sb = p_lat.tile([128, SC, DSL], BF16)     # v, [seq-chunk, d]
            wo_sb = p_wo.tile([128, HPC, HIDDEN], F32R)
            mask_sb = p_const.tile([128, 2, 256], F32)
            ones_sb = p_const.tile([128, 1], BF16)
            ones8_sb = p_const.tile([128, 2, 16], FP8)
            bias_sb = p_const.tile([128, 1], F32)
            wvu_sb = p_const.tile([128, KC_L, DSL], BF16)

            # ============ phase A+Q: kv-latent shard + direct q ============
            with tc.tile_pool(name="p_rope", bufs=1) as p_rope, \
                 tc.tile_pool(name="p_rt", bufs=3) as p_rt, \
                 tc.tile_pool(name="ps_q", bufs=2, space="PSUM") as ps_q, \
                 tc.tile_pool(name="ps_pr", bufs=2, space="PSUM") as ps_pr:
              cos_sb = p_rope.tile([128, S], BF16)
              sin_sb = p_rope.tile([128, S], BF16)
              perm_sb = p_rope.tile([128, 128], BF16)
              wqe_sb = p_rope.tile([128, KC_H, DSL], BF16)

              def rope_block(dstT, h, sl, pa, add_engine=None):
                  """dstT[:, h, sl] = cos*pa + sin*perm(pa): rotate_half as
                  a signed-permutation matmul (no PE transposes)."""
                  raw = p_rt.tile([128, 512], BF16, tag="raw")
                  nc.scalar.copy(raw, pa)
                  pr = ps_pr.tile([128, 512], F32, tag="pr")
                  nc.tensor.matmul(pr, perm_sb, raw, start=True, stop=True)
                  rawp = p_rt.tile([128, 512], BF16, tag="rawp")
                  nc.scalar.copy(rawp, pr)
                  rt = p_rt.tile([128, 512], BF16, tag="rt")
                  nc.vector.tensor_mul(dstT[:, h, sl], raw, cos_sb[:, sl])
                  nc.vector.tensor_mul(rt, rawp, sin_sb[:, sl])
                  eng = add_engine or nc.vector
                  eng.tensor_add(dstT[:, h, sl], dstT[:, h, sl], rt)

              # --- kv-local: latkv for this core's seq quarter -> DRAM ---
              wkvd3 = p_rope.tile([128, KC_H, 128], BF16)  # m3 column, Q-pass
              wku_sb = p_rope.tile([128, KC_L, DSL], BF16)
              _qes = ExitStack()
              p_xs = _qes.enter_context(tc.tile_pool(name="p_xs", bufs=2))
              with tc.tile_pool(name="p_wkvd", bufs=1) as p_wkvd, \
                   tc.tile_pool(name="p_xloc", bufs=1) as p_xloc, \
                   tc.tile_pool(name="p_st", bufs=2) as p_st, \
                   tc.tile_pool(name="ps_a", bufs=2, space="PSUM") as ps_a:
                  wkvd_sb = p_wkvd.tile([128, KC_H, MG * 128], BF16)
                  xloc = p_xloc.tile([128, KC_H, SQ], BF16)
                  # fine-grained first loads so the PE can start early
                  for kq in range(8):
                      ksl = slice(kq * 2, (kq + 1) * 2)
                      nc.sync.dma_start(
                          out=wkvd_sb[:, ksl, 0:128],
                          in_=wkvd.rearrange("(kc p) l -> p kc l", p=128)
                          [:, ksl, 0:128])
                      nc.scalar.dma_start(
                          out=xloc[:, ksl, :],
                          in_=xq.rearrange("(kc p) s -> p kc s", p=128)
                          [:, ksl, :])
                  for m in range(1, MG):
                      nc.sync.dma_start(
                          out=wkvd_sb[:, :, m * 128:(m + 1) * 128],
                          in_=wkvd.rearrange("(kc p) l -> p kc l", p=128)
                          [:, :, m * 128:(m + 1) * 128])
                  for kq in range(4):
                      ksl = slice(kq * 4, (kq + 1) * 4)
                      nc.scalar.dma_start(
                          out=wqe_sb[:, ksl, :],
                          in_=wqe.rearrange("(kc p) d -> p kc d", p=128)
                          [:, ksl, :])
                  nc.scalar.dma_start(
                      out=wkvd3,
                      in_=wkvd.rearrange("(kc p) l -> p kc l", p=128)
                      [:, :, MG * 128:KC_L * 128])
                  nc.scalar.dma_start(out=cos_sb, in_=cosd[:, :])
                  nc.scalar.dma_start(out=sin_sb, in_=sind[:, :])
                  nc.scalar.dma_start(out=perm_sb, in_=permd[:, :])
                  for m in range(MG):
                      acc = ps_a.tile([128, SQ], F32, tag="acc_a")
                      for kc in range(KC_H):
                          nc.tensor.matmul(
                              acc, wkvd_sb[:, kc, m * 128:(m + 1) * 128],
                              xloc[:, kc, :],
                              start=(kc == 0), stop=(kc == KC_H - 1))
                      st = p_st.tile([128, SQ], BF16, tag="st")
                      nc.vector.tensor_copy(st, acc)
                      nc.scalar.dma_start(out=latpkv[:, m, :], in_=st)

              # --- AllGather of the kv latents (overlaps the q phase) ---
              nc.gpsimd.collective_compute(
                  "AllGather", mybir.AluOpType.bypass,
                  replica_groups=[[0, 1, 2, 3], [4, 5, 6, 7]],
                  ins=[latpkv[:, :, :]], outs=[gkv[:, :, :, :]])

              # --- q: direct projection + rope, per seq block x head ---
              if True:
                  for nb in range(NB):
                      xslab = p_xs.tile([128, KC_H, 512], BF16, tag="xslab")
                      nc.sync.dma_start(
                          out=xslab,
                          in_=xT.rearrange("(kc p) s -> p kc s", p=128)
                          [:, :, nb * 512:(nb + 1) * 512])
                      if nb == 1:
                          nc.sync.dma_start(
                              out=wku_sb,
                              in_=wku.rearrange("(kc p) d -> p kc d", p=128))
                      if nb == 2:
                          nc.sync.dma_start(
                              out=wvu_sb,
                              in_=wvu.rearrange("(kc p) d -> p kc d", p=128))
                      for h in range(HPC):
                          pa = ps_q.tile([128, 512], F32, tag="pa")
                          for kc in range(KC_H):
                              nc.tensor.matmul(
                                  pa, wqe_sb[:, kc, h * 128:(h + 1) * 128],
                                  xslab[:, kc, :],
                                  start=(kc == 0), stop=(kc == KC_H - 1))
                          rope_block(qTr, h,
                                     slice(nb * 512, (nb + 1) * 512), pa)
                      # m3 kv-latent chunk for this slab (full S, local)
                      pm = ps_q.tile([128, 512], F32, tag="pa")
                      for kc in range(KC_H):
                          nc.tensor.matmul(
                              pm, wkvd3[:, kc, :], xslab[:, kc, :],
                              start=(kc == 0), stop=(kc == KC_H - 1))
                      nc.vector.tensor_copy(
                          latkv[:, KC_L - 1, nb * 512:(nb + 1) * 512], pm)

              _qes.close()
              # gathered latents back to SBUF, chunked per (m, t); these
              # depend on the AllGather, so issue them on the Pool queue
              # (which is busy with the collective anyway) to keep SP/ACT
              # free for the compute-feeding loads.
              for m in range(MG):
                  for t in range(TP):
                      tsl = slice(t * SQ, (t + 1) * SQ)
                      nc.gpsimd.dma_start(
                          out=latkv[:, m, tsl], in_=gkv[t, :, m, :])
              nc.gpsimd.dma_start(
                  out=mask_sb, in_=mtd.rearrange("j p c -> p j c"))
              nc.gpsimd.dma_start(out=ones_sb, in_=onesd[:, :])
              nc.gpsimd.dma_start(out=ones8_sb, in_=onesd8[:, :, :])
              nc.gpsimd.dma_start(out=bias_sb, in_=biasd[:, :])
              nc.sync.dma_start(
                  out=wo_sb, in_=wo.rearrange("(ic p) o -> p ic o", p=128))


              # --- k up-proj + rope interleaved with v chains ---
              with tc.tile_pool(name="ps_v", bufs=2, space="PSUM") as ps_v, \
                   tc.tile_pool(name="ps_k", bufs=2, space="PSUM") as ps_k:
                  for h in range(HPC):
                      for nb in range(NB):
                          pa = ps_k.tile([128, 512], F32, tag="pk")
                          for kc in range(KC_L):
                              nc.tensor.matmul(
                                  pa, wku_sb[:, kc, h * 128:(h + 1) * 128],
                                  latkv[:, kc, nb * 512:(nb + 1) * 512],
                                  start=(kc == 0), stop=(kc == KC_L - 1))
                          rope_block(kTr, h,
                                     slice(nb * 512, (nb + 1) * 512), pa)
                          sc = h * 4 + nb
                          acc = ps_v.tile([128, DSL], F32, tag="acc_v")
                          for kc in range(KC_L):
                              nc.tensor.matmul(
                                  acc,
                                  latkv[:, kc, sc * 128:(sc + 1) * 128],
                                  wvu_sb[:, kc, :],
                                  start=(kc == 0), stop=(kc == KC_L - 1))
                          if sc % 2 == 0:
                              nc.vector.tensor_copy(v_sb[:, sc, :], acc)
                          else:
                              nc.scalar.copy(v_sb[:, sc, :], acc)

            # ================= attention =================
            with tc.tile_pool(name="p_out", bufs=1) as p_out:
              outT = p_out.tile([128, HPC, S], F32R)  # attention out
              # ================= attention per head =================
              with tc.tile_pool(name="p_at", bufs=12) as p_at, \
                   tc.tile_pool(name="p_rb", bufs=2) as p_rb, \
                   tc.tile_pool(name="ps_s", bufs=2, space="PSUM") as ps_s, \
                   tc.tile_pool(name="ps_o", bufs=3, space="PSUM") as ps_o, \
                   tc.tile_pool(name="ps_n", bufs=1, space="PSUM") as ps_n:
                def emit_scores(h, qb):
                    qsl = slice(qb * 512, (qb + 1) * 512)
                    ets = []
                    for kp in range(2 * qb):
                        ps2 = ps_s.tile([128, 2, 512], F32, tag="ps")
                        for i in range(2):
                            kb = 2 * kp + i
                            nc.tensor.matmul(
                                ps2[:, i, :],
                                kTr[:, h, kb * 128:(kb + 1) * 128],
                                qTr[:, h, qsl],
                                start=True, stop=True)
                        et2 = p_at.tile([128, 2, 512], FP8, tag="et")
                        nc.scalar.activation(
                            out=et2, in_=ps2,
                            func=mybir.ActivationFunctionType.Exp,
                            scale=float(SCALE), bias=bias_sb[:, 0:1])
                        ets.append(et2)
                    etds = []
                    for j in range(4):
                        off = min(j, 2) * 128
                        w = 512 - off
                        q0 = qb * 512 + off
                        kb = 4 * qb + j
                        ps2 = ps_s.tile([128, 2, 512], F32, tag="ps")
                        nc.tensor.matmul(
                            ps2[:, 0, 0:w],
                            kTr[:, h, kb * 128:(kb + 1) * 128],
                            qTr[:, h, q0:q0 + w],
                            start=True, stop=True)
                        jj = j - off // 128
                        mw = (jj + 1) * 128
                        nc.vector.tensor_add(
                            ps2[:, 0, 0:mw], ps2[:, 0, 0:mw],
                            mask_sb[:, jj, 0:mw])
                        etd = p_at.tile([128, 512], BF16, tag="etd")
                        nc.scalar.activation(
                            out=etd[:, 0:w], in_=ps2[:, 0, 0:w],
                            func=mybir.ActivationFunctionType.Exp,
                            scale=float(SCALE), bias=bias_sb[:, 0:1])
                        etds.append(etd)
                    return h, qb, ets, etds

                def emit_av(unit):
                    h, qb, ets, etds = unit
                    hs = h * 128
                    qsl = slice(qb * 512, (qb + 1) * 512)
                    po = ps_o.tile([128, 512], F32, tag="po")
                    pn = ps_n.tile([1, 512], F32, tag="pn")
                    for kp, et2 in enumerate(ets):
                        for i in range(2):
                            kb = 2 * kp + i
                            nc.tensor.matmul(
                                po,
                                v_sb[:, kb, hs:hs + 128],
                                et2[:, i, :],
                                start=(kb == 0), stop=False)
                        nc.tensor.matmul(
                            pn[0:1, :],
                            ones8_sb[:, :, 0:1],
                            et2[:, :, :],
                            start=(kp == 0), stop=False,
                            perf_mode=mybir.MatmulPerfMode.DoubleRow)
                    for j, etd in enumerate(etds):
                        off = min(j, 2) * 128
                        w = 512 - off
                        kb = 4 * qb + j
                        nc.tensor.matmul(
                            po[:, off:512],
                            v_sb[:, kb, hs:hs + 128],
                            etd[:, 0:w],
                            start=(kb == 0), stop=(j == 3))
                        nc.tensor.matmul(
                            pn[0:1, off:512],
                            ones_sb[:, 0:1],
                            etd[:, 0:w],
                            start=(qb == 0 and j == 0), stop=(j == 3))
                    rc = p_rb.tile([1, 512], F32, tag="rc")
                    nc.vector.reciprocal(rc, pn[0:1, :])
                    rb = p_rb.tile([128, 512], F32, tag="rb")
                    nc.gpsimd.partition_broadcast(rb, rc)
                    nc.vector.tensor_mul(outT[:, h, qsl], po, rb)

                prev = None
                for h in range(HPC):
                    for qb in range(NB):
                        cur = emit_scores(h, qb)
                        if prev is not None:
                            emit_av(prev)
                        prev = cur
                emit_av(prev)

            # ================= output projection =================
              with tc.tile_pool(name="p_fin", bufs=3) as p_fin, \
                   tc.tile_pool(name="ps_d", bufs=4, space="PSUM") as ps_d:
                for sc in range(SC):
                    fin = p_fin.tile([128, HIDDEN], F32, tag="fin")
                    for ob in range(HIDDEN // 512):
                        acc = ps_d.tile([128, 512], F32, tag="acc_d")
                        for ic in range(HPC):
                            nc.tensor.matmul(
                                acc,
                                outT[:, ic, sc * 128:(sc + 1) * 128],
                                wo_sb[:, ic, ob * 512:(ob + 1) * 512],
                                start=(ic == 0), stop=(ic == HPC - 1))
                        osl = slice(ob * 512, (ob + 1) * 512)
                        if ob % 2 == 0:
                            nc.scalar.copy(fin[:, osl], acc)
                        else:
                            nc.vector.tensor_copy(fin[:, osl], acc)
                    if sc == SC - 1:
                        for oq in range(4):
                            osl4 = slice(oq * 512, (oq + 1) * 512)
                            nc.sync.dma_start(
                                out=y[sc * 128:(sc + 1) * 128, osl4],
                                in_=fin[:, osl4])
                    else:
                        nc.sync.dma_start(
                            out=y[sc * 128:(sc + 1) * 128, :], in_=fin)

    if finalize:
        nc.finalize()
    return nc


# ---------------------------------------------------------------------------
# host-side helpers


def host_inputs(x, Wq_d, Wkv_d, Wq_u, Wk_u, Wv_u, Wo, S=S_FULL, gather=True):
    """Build the 8 per-core input maps from full inputs."""
    import ml_dtypes
    bf16 = ml_dtypes.bfloat16

    x = np.asarray(x, dtype=np.float32)
    Wq_d = np.asarray(Wq_d, dtype=np.float64)
    Wkv_d = np.asarray(Wkv_d, dtype=np.float32)
    Wq_u = np.asarray(Wq_u, dtype=np.float64)
    Wk_u = np.asarray(Wk_u, dtype=np.float32)
    Wv_u = np.asarray(Wv_u, dtype=np.float32)
    Wo = np.asarray(Wo, dtype=np.float32)

    inv_freq = 1.0 / (THETA ** (np.arange(0, HEAD_DIM, 2, dtype=np.float64)
                                / HEAD_DIM))  # (64,)
    pos = np.arange(S, dtype=np.float64)
    ang = pos[None, :] * np.concatenate([inv_freq, inv_freq])[:, None]  # (128, S)
    COS = np.cos(ang).astype(bf16)
    SIN = np.sin(ang).astype(bf16)

    # signed permutation for rotate_half in [d, seq] layout:
    # out[m] = -in[m+64] for m<64 ; +in[m-64] for m>=64
    PERM = np.zeros((128, 128), dtype=np.float32)
    for m in range(64):
        PERM[m + 64, m] = -1.0
        PERM[m, m + 64] = 1.0

    # masks for narrowed diagonal blocks: jj=0 -> mask c<r on first 128 cols;
    # jj=1 -> mask c<128+r on first 256 cols
    MT = np.zeros((2, 128, 256), dtype=np.float32)
    r = np.arange(128)[:, None]
    c = np.arange(256)[None, :]
    MT[0] = np.where(c >= r, 0.0, NEG)
    MT[1] = np.where(c >= 128 + r, 0.0, NEG)

    in_maps = []
    for core in range(N_CORES):
        b, tp = core // TP, core % TP
        sl = slice(tp * DSL, (tp + 1) * DSL)
        SQ = S // TP
        xt_c = np.ascontiguousarray(x[b, :S].T.astype(bf16))
        wqe = (Wq_d @ Wq_u[:, sl]).astype(bf16)
        in_maps.append({
            "xT": xt_c,
            "xq": np.ascontiguousarray(xt_c[:, tp * SQ:(tp + 1) * SQ]),
            "wkvd": Wkv_d.astype(bf16),
            "wqe": np.ascontiguousarray(wqe),
            "wku": np.ascontiguousarray(Wk_u[:, sl].astype(bf16)),
            "wvu": np.ascontiguousarray(Wv_u[:, sl].astype(bf16)),
            "wo": np.ascontiguousarray(Wo[sl, :]),
            "cosd": COS,
            "sind": SIN,
            "mtd": MT,
            "onesd": np.ones((128, 1), dtype=ml_dtypes.bfloat16),
            "onesd8": np.ones((128, 2, 16), dtype=ml_dtypes.float8_e4m3fn),
            "permd": PERM.astype(bf16),
            "biasd": np.full((128, 1), EXP_BIAS, dtype=np.float32),
        })
    return in_maps


def assemble(results, S=S_FULL):
    out = np.zeros((B, S, HIDDEN), dtype=np.float32)
    for core in range(N_CORES):
        out[core // TP] += results[core]["y"]
    return out


_NC_CACHE = {}


def kernel(x, Wq_d, Wkv_d, Wq_u, Wk_u, Wv_u, Wo):
    S = x.shape[1]
    if S not in _NC_CACHE:
        _NC_CACHE[S] = build_nc(S)
    nc = _NC_CACHE[S]
    in_maps = host_inputs(x, Wq_d, Wkv_d, Wq_u, Wk_u, Wv_u, Wo, S=S)

    res = run_bass_kernel_spmd(nc, in_maps, list(range(N_CORES)))
    return assemble(res.results, S=S)
